# revision 18
# baseline (speedup 1.0000x reference)
"""Trainium2 Bass kernel for nn_MultiHeadLayer (pre-LN MHA, fused QKV).

Self-contained: takes FULL inputs, shards data-parallel over batch across
8 NeuronCores, runs a Bass/Tile kernel per core, gathers the full output.

Per-core dataflow (T = B_core*S tokens, H hidden, NH heads, D = H/NH),
all matmul operands bf16 (1.0 PE cycles/row, same rate as f32r, half the
HBM traffic), fp32 PSUM accumulation, everything SBUF-resident (no DRAM
scratch round-trip):

  Phase A: LN in natural layout (var = E[x^2]-mu^2 so the DVE reduce and
           the ACT square run concurrently) -> PE-transpose ->
           xnT [H, T] bf16 in SBUF.
  Heads (n = 0..NH-1), fully fused:
    qk-proj: weight-stationary matmuls -> qT,kT [D, T] bf16 (1/sqrt(D)
             pre-folded into the q columns on host)
    v-proj:  activation-stationary matmuls (xnT chunks stationary, Wv
             moving) -> v [T, D] bf16 -- directly in the layout the
             ctx matmul needs, so NO transposes in attention
    attention per batch: scoresT = kT.T@qT (PSUM), exp fused with the
             additive key mask via per-partition ACT bias (no max
             subtraction: |scores| <~ 40 << 88), sumexp via ones-matmul,
             ctxT accumulated into SBUF with the 1/sumexp fused into the
             PSUM evacuation. Software-pipelined one batch ahead so the
             PE never waits on the ACT exp.
  O-proj:  weight-stationary -> outT [H, T] bf16 -> host transposes.
           Emission is interleaved with the NEXT repeat's Phase A so the
           LN (DVE/ACT) hides under the O-proj matmuls in steady state.
"""

import numpy as np
from functools import lru_cache

LN_EPS = 1e-5
NEG_BIG = -1.0e30


def _build(n_cores, T, S, H, NH, is_pre, has_bias, repeat=1):
    import os
    import concourse.bacc as bacc
    import concourse.mybir as mybir
    import concourse.tile as tile
    from concourse.masks import make_identity

    # timing-only probe: load each weight tile once and reuse it for all
    # later chunks (outputs become garbage; never set in real use)
    probe_no_wdma = bool(int(os.environ.get("PROBE_NO_WDMA", "0")))
    _wcache = {}

    F32 = mybir.dt.float32
    F32R = mybir.dt.float32r
    BF16 = mybir.dt.bfloat16
    ACT = mybir.ActivationFunctionType
    ALU = mybir.AluOpType

    KO = H // 128          # hidden-dim 128-chunks (contraction)
    D = H // NH
    DT = D // 128          # d-chunks per head
    KT = S // 128          # key-token 128-chunks per sequence
    B_core = T // S
    TT = T // 128          # token 128-chunks
    TC = T // 512          # token 512-chunks
    NC2 = 2 * H // 128     # q+k column chunks of 128
    assert TC >= 1 and DT == 2 and KT == 2

    nc = bacc.Bacc("TRN2", target_bir_lowering=False, debug=False,
                   num_devices=n_cores)

    x_d = nc.dram_tensor("x", [T, H], F32, kind="ExternalInput")
    # host prepacks weights so each SBUF tile is one linear read:
    # qkw[c, p, ko, j]  = Wqk[ko*128+p, c*128+j]        (c < NC2)
    # vw [n, p, ko, j]  = Wv [ko*128+p, n*D+j]          (n < NH, j < D)
    # ow [c, p, ko, j]  = Wo [ko*128+p, c*128+j]        (c < KO)
    qkw_d = nc.dram_tensor("qkw", [NC2 // 2, 128, KO, 256], BF16,
                           kind="ExternalInput")
    vw_d = nc.dram_tensor("vw", [NH, 128, KO, D], BF16,
                          kind="ExternalInput")
    ow_d = nc.dram_tensor("ow", [KO // 2, 128, KO, 256], BF16,
                          kind="ExternalInput")
    # maskb[b*KT+kt, :] = additive key-mask bias for key tokens kt*128..+128
    mb_d = nc.dram_tensor("maskb", [B_core * KT, 128], F32,
                          kind="ExternalInput")
    if has_bias:
        # bqk[i, :] = (bias @ qkv)[i*128:(i+1)*128] for the q,k columns
        # (q part pre-scaled by 1/sqrt(D)); bv broadcast row for v columns.
        bqk_d = nc.dram_tensor("bqk", [NC2, 128], F32, kind="ExternalInput")
        bv_d = nc.dram_tensor("bv", [H], F32, kind="ExternalInput")
    if is_pre:
        out_d = nc.dram_tensor("outT", [H, T], BF16, kind="ExternalOutput")
    else:
        lnw_d = nc.dram_tensor("lnw", [H], F32, kind="ExternalInput")
        lnb_d = nc.dram_tensor("lnb", [H], F32, kind="ExternalInput")
        out_d = nc.dram_tensor("outN", [T, H], F32, kind="ExternalOutput")

    from contextlib import ExitStack

    with tile.TileContext(nc) as tc:
        with ExitStack() as stack:
            pool = lambda *a, **kw: stack.enter_context(  # noqa: E731
                tc.tile_pool(*a, **kw))
            cp = pool(name="consts", bufs=1)
            dp = pool(name="dram", bufs=1, space="DRAM")
            xp = pool(name="xld", bufs=1)
            xbp = pool(name="xnb", bufs=1)
            st = pool(name="stats", bufs=4)
            xnp = pool(name="xnt", bufs=1)
            wp = pool(name="wqk", bufs=2)
            # bufs=1 is safe for qkh/vhd: PE executes in order, so head n's
            # attention reads finish before head n+1's projection evacs.
            qkp = pool(name="qkh", bufs=1)
            vhp = pool(name="vhd", bufs=1)
            xpp = pool(name="expp", bufs=2)
            rp = pool(name="recp", bufs=2)
            oev = pool(name="oevp", bufs=1)
            cxp = pool(name="ctxp", bufs=1)
            # PSUM is 8 banks of 2KB/partition; every tile rounds up to a
            # bank: pQK 2 + pS 2 (transposes borrow it) + pV 2 + m 1 + c 1.
            pQK = pool(name="psQK", bufs=2, space="PSUM")
            pV = pool(name="psV", bufs=2, space="PSUM")
            pS = pool(name="psS", bufs=2, space="PSUM")
            pMC = pool(name="psMC", bufs=1, space="PSUM")

            if not is_pre:
                ident = cp.tile([128, 128], F32)
                make_identity(nc, ident[:])
            onesb = cp.tile([128, 128], BF16)
            # f32 bit pattern 0x3F803F80 = two bf16 1.0s per 4-byte word
            nc.vector.memset(onesb[:].bitcast(F32),
                             float(np.frombuffer(
                                 np.uint32(0x3F803F80).tobytes(),
                                 dtype=np.float32)[0]))
            eps_t = cp.tile([128, 1], F32)
            nc.vector.memset(eps_t[:], LN_EPS)
            mb_t = cp.tile([128, B_core * KT], F32)
            nc.sync.dma_start(mb_t[:], mb_d.ap().rearrange("i p -> p i"))
            if has_bias:
                bqk_t = cp.tile([128, NC2], F32)
                nc.sync.dma_start(bqk_t[:], bqk_d.ap().rearrange("i p -> p i"))
                import concourse.bass as _bass
                bv_bc = _bass.AP(tensor=bv_d.ap().tensor, offset=0,
                                 ap=[[0, 128], [1, H]])
                bv_t = cp.tile([128, H], F32)
                nc.sync.dma_start(bv_t[:], bv_bc)
            if not is_pre:
                import concourse.bass as _bass
                lnw_bc = _bass.AP(tensor=lnw_d.ap().tensor, offset=0,
                                  ap=[[0, 128], [1, H]])
                lnb_bc = _bass.AP(tensor=lnb_d.ap().tensor, offset=0,
                                  ap=[[0, 128], [1, H]])
                lnw_t = cp.tile([128, H], F32)
                nc.sync.dma_start(lnw_t[:], lnw_bc)
                lnb_t = cp.tile([128, H], F32)
                nc.sync.dma_start(lnb_t[:], lnb_bc)
                oTs = dp.tile([H, T], F32)

            def _wtile(pool_, shape, tag, src_ap):
                if probe_no_wdma and tag in _wcache:
                    return _wcache[tag]
                t = pool_.tile(shape, BF16, tag=tag, name=tag)
                nc.sync.dma_start(t[:], src_ap)
                if probe_no_wdma:
                    _wcache[tag] = t
                return t

            def emit_phaseA_tt(xnT, tt):
                """LN one 128-token block; DMA-XBAR-transpose into xnT."""
                xt = xp.tile([128, H], F32, tag="xt", name="xt")
                nc.sync.dma_start(xt[:], x_d.ap()[tt * 128:(tt + 1) * 128, :])
                xnb = xbp.tile([128, H // 2], BF16, tag="xnb", name="xnb")
                if is_pre:
                    ssum = st.tile([128, 1], F32, tag="ss", name="ss")
                    nc.vector.reduce_sum(out=ssum[:], in_=xt[:],
                                         axis=mybir.AxisListType.X)
                    # sum of squares, chunked; the elementwise output is
                    # discarded scratch (xnb is overwritten below)
                    NSQ = 4
                    HC = H // NSQ
                    parts = []
                    for i in range(NSQ):
                        p = st.tile([128, 1], F32, tag=f"sq{i}",
                                    name=f"sq{i}")
                        nc.scalar.activation(
                            xnb[:, (i % 2) * HC:(i % 2 + 1) * HC],
                            xt[:, i * HC:(i + 1) * HC],
                            ACT.Square, accum_out=p[:])
                        parts.append(p)
                    while len(parts) > 1:
                        a = parts.pop(0)
                        b_ = parts.pop(0)
                        s = st.tile([128, 1], F32, tag="sqa", name="sqa")
                        nc.vector.tensor_tensor(s[:], a[:], b_[:], ALU.add)
                        parts.append(s)
                    sumsq = parts[0]
                    negmu = st.tile([128, 1], F32, tag="nm", name="nm")
                    nc.vector.tensor_scalar_mul(negmu[:], ssum[:], -1.0 / H)
                    musq = st.tile([128, 1], F32, tag="mq", name="mq")
                    nc.vector.tensor_tensor(musq[:], negmu[:], negmu[:],
                                            ALU.mult)
                    bias_t = st.tile([128, 1], F32, tag="bt", name="bt")
                    nc.vector.tensor_tensor(bias_t[:], eps_t[:], musq[:],
                                            ALU.subtract)
                    sd = st.tile([128, 1], F32, tag="sd", name="sd")
                    nc.scalar.activation(sd[:], sumsq[:], ACT.Sqrt,
                                         bias=bias_t[:], scale=1.0 / H)
                    rstd = st.tile([128, 1], F32, tag="rs", name="rs")
                    nc.vector.reciprocal(rstd[:], sd[:])
                for h in range(2):
                    dst = xnT[:, h * (KO // 2):(h + 1) * (KO // 2),
                              tt * 128:(tt + 1) * 128]
                    if is_pre:
                        nc.vector.tensor_scalar(
                            out=xnb[:], in0=xt[:, h * (H // 2):
                                               (h + 1) * (H // 2)],
                            scalar1=negmu[:], scalar2=rstd[:],
                            op0=ALU.add, op1=ALU.mult)
                    else:
                        nc.vector.tensor_copy(
                            xnb[:], xt[:, h * (H // 2):(h + 1) * (H // 2)])
                    nc.sync.dma_start_transpose(dst, xnb[:])

            def emit_head(xnT, ctxT, n):
                # ---- q,k projections (weight-stationary) ----
                # f32r: the softmax is sharp (score std ~6), so bf16 q/k
                # storage noise (~0.01 absolute on scores) costs ~1% output
                # error; f32r storage keeps scores near-exact and runs at
                # the same PE rate for free dim >= 256.
                qT = qkp.tile([128, DT, T], F32R, tag="qT", name="qT")
                kT = qkp.tile([128, DT, T], F32R, tag="kT", name="kT")
                for which, dest in ((0, qT), (1, kT)):
                    wt = _wtile(wp, [128, KO, 256], "w",
                                qkw_d.ap()[which * (NC2 // 4) + n])
                    for dt in range(DT):
                        c = which * (H // 128) + n * DT + dt
                        for tch in range(TC):
                            ps = pQK.tile([128, 512], F32, tag="qk",
                                          name="psqk")
                            for ko in range(KO):
                                nc.tensor.matmul(
                                    ps[:], wt[:, ko, dt * 128:(dt + 1) * 128],
                                    xnT[:, ko, tch * 512:(tch + 1) * 512],
                                    start=(ko == 0), stop=(ko == KO - 1))
                            dst = dest[:, dt, tch * 512:(tch + 1) * 512]
                            if has_bias:
                                nc.vector.tensor_scalar_add(
                                    dst, ps[:], bqk_t[:, c:c + 1])
                            else:
                                nc.vector.tensor_copy(dst, ps[:])
                # ---- v projection (activation-stationary) ----
                wv = _wtile(wp, [128, KO, D], "w", vw_d.ap()[n])
                vh = vhp.tile([128, TT, D], BF16, tag="vh", name="vh")
                for t8 in range(TT):
                    ps = pV.tile([128, D], F32, tag="v", name="psv")
                    for ko in range(KO):
                        nc.tensor.matmul(
                            ps[:], xnT[:, ko, t8 * 128:(t8 + 1) * 128],
                            wv[:, ko], start=(ko == 0), stop=(ko == KO - 1))
                    if has_bias:
                        nc.vector.tensor_tensor(
                            vh[:, t8], ps[:], bv_t[:, n * D:(n + 1) * D],
                            ALU.add)
                    else:
                        nc.vector.tensor_copy(vh[:, t8], ps[:])

                # ---- attention, software-pipelined one batch ahead ----
                pss = {}
                exps = {}

                def emit_scores(b):
                    ps = pS.tile([128, 2 * S], F32, tag="s", name="pss")
                    for kt in range(KT):
                        for dt in range(DT):
                            nc.tensor.matmul(
                                ps[:, kt * S:(kt + 1) * S],
                                kT[:, dt, b * S + kt * 128:
                                   b * S + (kt + 1) * 128],
                                qT[:, dt, b * S:(b + 1) * S],
                                start=(dt == 0), stop=(dt == DT - 1))
                    ex = xpp.tile([128, KT, S], BF16, tag="ex", name="ex")
                    for kt in range(KT):
                        nc.scalar.activation(
                            ex[:, kt], ps[:, kt * S:(kt + 1) * S], ACT.Exp,
                            bias=mb_t[:, b * KT + kt:b * KT + kt + 1],
                            scale=1.0)
                    pss[b] = ps
                    exps[b] = ex

                def emit_ctx(b):
                    ex = exps.pop(b)
                    pss.pop(b)
                    psm = pMC.tile([128, S], F32, tag="m", name="psm")
                    for kt in range(KT):
                        nc.tensor.matmul(psm[:], onesb[:], ex[:, kt],
                                         start=(kt == 0), stop=(kt == KT - 1))
                    rec = rp.tile([128, S], F32, tag="rec", name="rec")
                    nc.vector.reciprocal(rec[:], psm[:])
                    psc = pMC.tile([128, DT, S], F32, tag="c", name="psc")
                    for dt in range(DT):
                        for kt in range(KT):
                            nc.tensor.matmul(
                                psc[:, dt],
                                vh[:, 2 * b + kt, dt * 128:(dt + 1) * 128],
                                ex[:, kt, :],
                                start=(kt == 0), stop=(kt == KT - 1))
                        nc.vector.tensor_tensor(
                            ctxT[:, n * DT + dt, b * S:(b + 1) * S],
                            psc[:, dt], rec[:], ALU.mult)

                emit_scores(0)
                for b in range(B_core):
                    if b + 1 < B_core:
                        emit_scores(b + 1)
                    emit_ctx(b)

            def emit_oproj_chunk(ctxT, hoch2):
                wo = _wtile(wp, [128, KO, 256], "w", ow_d.ap()[hoch2])
                for half in range(2):
                    hoch = 2 * hoch2 + half
                    for tch in range(TC):
                        ps = pQK.tile([128, 512], F32, tag="qk", name="psqk")
                        for ko in range(KO):
                            nc.tensor.matmul(
                                ps[:], wo[:, ko, half * 128:(half + 1) * 128],
                                ctxT[:, ko, tch * 512:(tch + 1) * 512],
                                start=(ko == 0), stop=(ko == KO - 1))
                        if is_pre:
                            ev = oev.tile([128, 512], BF16, tag="ev",
                                          name="ev")
                            nc.vector.tensor_copy(ev[:], ps[:])
                            nc.sync.dma_start(
                                out_d.ap()[hoch * 128:(hoch + 1) * 128,
                                           tch * 512:(tch + 1) * 512], ev[:])
                        else:
                            ev = oev.tile([128, 512], F32, tag="ev",
                                          name="ev")
                            nc.vector.tensor_copy(ev[:], ps[:])
                            nc.sync.dma_start(
                                oTs[hoch * 128:(hoch + 1) * 128,
                                    tch * 512:(tch + 1) * 512], ev[:])

            # ---------------- main repeat loop ----------------
            xnT = xnp.tile([128, KO, T], BF16, tag="xnT", name="xnT")
            for tt in range(TT):
                emit_phaseA_tt(xnT, tt)
            for rep in range(repeat):
                ctxT = cxp.tile([128, KO, T], BF16, tag="ctxT", name="ctxT")
                for n in range(NH):
                    emit_head(xnT, ctxT, n)
                NO2 = KO // 2
                if rep + 1 < repeat:
                    # interleave O-proj with the next rep's Phase A so the
                    # LN DVE/ACT work hides under the O-proj matmuls.
                    xnT = xnp.tile([128, KO, T], BF16, tag="xnT", name="xnT")
                    for hoch2 in range(NO2):
                        emit_oproj_chunk(ctxT, hoch2)
                        if hoch2 % (NO2 // TT) == 0:
                            emit_phaseA_tt(xnT, hoch2 // (NO2 // TT))
                else:
                    for hoch2 in range(NO2):
                        emit_oproj_chunk(ctxT, hoch2)

            # ---------------- isPre=0: transpose + post-LN ----------------
            if not is_pre:
                with tc.tile_pool(name="p4in", bufs=3) as p4i, \
                     tc.tile_pool(name="p4out", bufs=2) as p4o, \
                     tc.tile_pool(name="st4", bufs=8) as st4, \
                     tc.tile_pool(name="sq4", bufs=2) as sq4, \
                     tc.tile_pool(name="tps4", bufs=4, space="PSUM") as tp4:
                    for tt in range(TT):
                        on = p4o.tile([128, H], F32)
                        for hh in range(KO):
                            it = p4i.tile([128, 128], F32)
                            nc.sync.dma_start(
                                it[:], oTs[hh * 128:(hh + 1) * 128,
                                           tt * 128:(tt + 1) * 128])
                            pt = tp4.tile([128, 128], F32)
                            nc.tensor.transpose(pt[:], it[:], ident[:])
                            nc.vector.tensor_copy(
                                on[:, hh * 128:(hh + 1) * 128], pt[:])
                        ssum = st4.tile([128, 1], F32)
                        nc.vector.reduce_sum(out=ssum[:], in_=on[:],
                                             axis=mybir.AxisListType.X)
                        negmu = st4.tile([128, 1], F32)
                        nc.vector.tensor_scalar_mul(negmu[:], ssum[:],
                                                    -1.0 / H)
                        xsq = sq4.tile([128, H], F32)
                        vsum = st4.tile([128, 1], F32)
                        nc.scalar.activation(xsq[:], on[:], ACT.Square,
                                             bias=negmu[:], scale=1.0,
                                             accum_out=vsum[:])
                        sd = st4.tile([128, 1], F32)
                        nc.scalar.activation(sd[:], vsum[:], ACT.Sqrt,
                                             bias=eps_t[:], scale=1.0 / H)
                        rstd = st4.tile([128, 1], F32)
                        nc.vector.reciprocal(rstd[:], sd[:])
                        nc.vector.tensor_scalar(
                            out=on[:], in0=on[:],
                            scalar1=negmu[:], scalar2=rstd[:],
                            op0=ALU.add, op1=ALU.mult)
                        nc.vector.tensor_tensor(on[:], on[:], lnw_t[:],
                                                ALU.mult)
                        nc.vector.tensor_tensor(on[:], on[:], lnb_t[:],
                                                ALU.add)
                        nc.sync.dma_start(
                            out_d.ap()[tt * 128:(tt + 1) * 128, :], on[:])

    nc.finalize()
    return nc


@lru_cache(maxsize=4)
def _get_runner(n_cores, T, S, H, NH, is_pre, has_bias, repeat=1):
    """Build + jit once; returns fn(in_maps) -> list of out dicts."""
    import jax
    import numpy as _np
    from jax.sharding import Mesh, PartitionSpec
    from jax.experimental.shard_map import shard_map
    import concourse.mybir as mybir
    from concourse import bass2jax
    from concourse.bass2jax import _bass_exec_p, install_neuronx_cc_hook

    nc = _build(n_cores, T, S, H, NH, is_pre, has_bias, repeat)
    install_neuronx_cc_hook()

    partition_name = (nc.partition_id_tensor.name
                      if nc.partition_id_tensor else None)
    in_names, out_names, out_avals, zero_shapes = [], [], [], []
    for alloc in nc.m.functions[0].allocations:
        if not isinstance(alloc, mybir.MemoryLocationSet):
            continue
        name = alloc.memorylocations[0].name
        if alloc.kind == "ExternalInput":
            if name != partition_name:
                in_names.append(name)
        elif alloc.kind == "ExternalOutput":
            out_names.append(name)
            shape = tuple(alloc.tensor_shape)
            dtype = mybir.dt.np(alloc.dtype)
            out_avals.append(jax.core.ShapedArray(shape, dtype))
            zero_shapes.append((shape, dtype))
    n_outs = len(out_avals)
    all_in_names = list(in_names) + list(out_names)
    if partition_name is not None:
        all_in_names.append(partition_name)

    def _body(*args):
        operands = list(args)
        if partition_name is not None:
            operands.append(bass2jax.partition_id_tensor())
        outs = _bass_exec_p.bind(
            *operands,
            out_avals=tuple(out_avals),
            in_names=tuple(all_in_names),
            out_names=tuple(out_names),
            lowering_input_output_aliases=(),
            sim_require_finite=True,
            sim_require_nnan=True,
            nc=nc,
        )
        return tuple(outs)

    devices = jax.devices()[:n_cores]
    if n_cores == 1:
        jfn = jax.jit(_body, keep_unused=True)

        def _prep(in_maps):
            args = [jax.device_put(_np.asarray(in_maps[0][n]))
                    for n in in_names]
            zeros = [jax.device_put(_np.zeros(s, d)) for s, d in zero_shapes]
            return args + zeros

        def _collect(outs):
            return [{n: _np.asarray(outs[i]) for i, n in enumerate(out_names)}]
    else:
        mesh = Mesh(np.asarray(devices), ("core",))
        from jax.sharding import NamedSharding
        shard = NamedSharding(mesh, PartitionSpec("core"))
        repl = NamedSharding(mesh, PartitionSpec())
        REPLICATED = {"qkw", "vw", "ow", "bqk", "bv", "lnw", "lnb"}
        in_specs = tuple(
            (PartitionSpec() if n in REPLICATED else PartitionSpec("core"))
            for n in in_names) + (PartitionSpec("core"),) * n_outs
        out_specs = (PartitionSpec("core"),) * n_outs
        jfn = jax.jit(
            shard_map(_body, mesh=mesh, in_specs=in_specs,
                      out_specs=out_specs, check_rep=False),
            keep_unused=True)

        def _prep(in_maps):
            concat_in = []
            for n in in_names:
                if n in REPLICATED:
                    concat_in.append(
                        jax.device_put(_np.asarray(in_maps[0][n]), repl))
                else:
                    concat_in.append(jax.device_put(
                        _np.concatenate([_np.asarray(m[n]) for m in in_maps],
                                        axis=0), shard))
            zeros = [
                jax.device_put(
                    _np.zeros((n_cores * s[0], *s[1:]), d), shard)
                for s, d in zero_shapes]
            return concat_in + zeros

        def _collect(outs):
            return [
                {n: _np.asarray(outs[i]).reshape(
                    n_cores, *out_avals[i].shape)[c]
                 for i, n in enumerate(out_names)}
                for c in range(n_cores)]

    class Runner:
        in_names_ = in_names
        out_names_ = out_names

        def prep(self, in_maps):
            return _prep(in_maps)

        def call(self, args):
            return jfn(*args)

        def run(self, in_maps):
            outs = jfn(*_prep(in_maps))
            jax.block_until_ready(outs)
            return _collect(outs)

        def collect(self, outs):
            return _collect(outs)

    return Runner()


def _prep_core_inputs(inp, mask, weight, bias, qkv, o, is_pre, n_cores,
                      NH=16):
    """Host-side prep: fold LN weight + 1/sqrt(D) into qkv, prepack the
    weights into per-tile-contiguous bf16 layouts, build per-core dicts."""
    import ml_dtypes
    BF16 = ml_dtypes.bfloat16

    B, S, H = inp.shape
    D = H // NH
    B_core = B // n_cores
    T = B_core * S
    KO = H // 128
    KT = S // 128
    NC2 = 2 * H // 128

    # Pre-LN: xn = z*w + b with z the normalized input, so
    # xn @ qkv = z @ (w[:,None]*qkv) + (b @ qkv): fold w into the weights
    # and b into per-column additive terms applied on-device. The
    # 1/sqrt(D) query scale is folded into the q weight columns.
    qkvw = qkv.astype(np.float32)
    if is_pre:
        w = weight.astype(np.float32)
        if not np.all(w == 1.0):
            qkvw = qkvw * w[:, None]
        bfull = bias.astype(np.float32) @ qkv.astype(np.float32)
    else:
        bfull = np.zeros(3 * H, dtype=np.float32)
    qsc = np.float32(1.0 / np.sqrt(D))
    qkvw = qkvw.copy()
    qkvw[:, :H] *= qsc
    bfull = bfull.copy()
    bfull[:H] *= qsc
    has_bias = bool(np.any(bfull))

    # prepack: tile [128, KO, 256] contiguous per merged pair-chunk index
    wqk = qkvw[:, :2 * H].reshape(KO, 128, NC2 // 2, 256)
    wqk = np.ascontiguousarray(wqk.transpose(2, 1, 0, 3)).astype(BF16)
    wv = qkvw[:, 2 * H:].reshape(KO, 128, NH, D)
    wv = np.ascontiguousarray(wv.transpose(2, 1, 0, 3)).astype(BF16)
    ow = o.astype(np.float32).reshape(KO, 128, KO // 2, 256)
    ow = np.ascontiguousarray(ow.transpose(2, 1, 0, 3)).astype(BF16)

    maskbias = np.where(mask != 0, np.float32(NEG_BIG), np.float32(0.0))
    maskbias = maskbias.astype(np.float32)  # [B, S]

    in_maps = []
    for c in range(n_cores):
        xb = inp[c * B_core:(c + 1) * B_core].reshape(T, H)
        mb = maskbias[c * B_core:(c + 1) * B_core].reshape(B_core * KT, 128)
        m = {
            "x": np.ascontiguousarray(xb.astype(np.float32)),
            "qkw": wqk,
            "vw": wv,
            "ow": ow,
            "maskb": np.ascontiguousarray(mb),
        }
        if has_bias:
            m["bqk"] = np.ascontiguousarray(
                bfull[:2 * H].reshape(NC2, 128))
            m["bv"] = np.ascontiguousarray(bfull[2 * H:])
        if not is_pre:
            m["lnw"] = np.ascontiguousarray(weight.astype(np.float32))
            m["lnb"] = np.ascontiguousarray(bias.astype(np.float32))
        in_maps.append(m)
    return in_maps, has_bias, (B, S, H, NH, B_core, T)


def kernel(inp, mask, weight, bias, qkv, o, isPre):
    inp = np.asarray(inp)
    mask = np.asarray(mask)
    weight = np.asarray(weight)
    bias = np.asarray(bias)
    qkv = np.asarray(qkv)
    o = np.asarray(o)
    is_pre = bool(int(np.asarray(isPre)))

    n_cores = 8
    NH = 16
    in_maps, has_bias, (B, S, H, _, B_core, T) = _prep_core_inputs(
        inp, mask, weight, bias, qkv, o, is_pre, n_cores)

    runner = _get_runner(n_cores, T, S, H, NH, is_pre, has_bias)
    results = runner.run(in_maps)

    out = np.empty((B, S, H), dtype=np.float32)
    for c in range(n_cores):
        if is_pre:
            outT = results[c]["outT"].astype(np.float32)  # [H, T]
            out[c * B_core:(c + 1) * B_core] = outT.T.reshape(B_core, S, H)
        else:
            out[c * B_core:(c + 1) * B_core] = (
                results[c]["outN"].reshape(B_core, S, H))
    return out


# revision 20
# speedup vs baseline: 1.1002x; 1.1002x over previous
"""Trainium2 Bass kernel for nn_MultiHeadLayer (pre-LN MHA, fused QKV).

Self-contained: takes FULL inputs, shards data-parallel over batch across
8 NeuronCores, runs a Bass/Tile kernel per core, gathers the full output.

Per-core dataflow (T = B_core*S tokens, H hidden, NH heads, D = H/NH),
all matmul operands bf16 (1.0 PE cycles/row, same rate as f32r, half the
HBM traffic), fp32 PSUM accumulation, everything SBUF-resident (no DRAM
scratch round-trip):

  Phase A: LN in natural layout (var = E[x^2]-mu^2 so the DVE reduce and
           the ACT square run concurrently) -> PE-transpose ->
           xnT [H, T] bf16 in SBUF.
  Heads (n = 0..NH-1), fully fused:
    qk-proj: weight-stationary matmuls -> qT,kT [D, T] bf16 (1/sqrt(D)
             pre-folded into the q columns on host)
    v-proj:  activation-stationary matmuls (xnT chunks stationary, Wv
             moving) -> v [T, D] bf16 -- directly in the layout the
             ctx matmul needs, so NO transposes in attention
    attention per batch: scoresT = kT.T@qT (PSUM), exp fused with the
             additive key mask via per-partition ACT bias (no max
             subtraction: |scores| <~ 40 << 88), sumexp via ones-matmul,
             ctxT accumulated into SBUF with the 1/sumexp fused into the
             PSUM evacuation. Software-pipelined one batch ahead so the
             PE never waits on the ACT exp.
  O-proj:  weight-stationary -> outT [H, T] bf16 -> host transposes.
           Emission is interleaved with the NEXT repeat's Phase A so the
           LN (DVE/ACT) hides under the O-proj matmuls in steady state.
"""

import numpy as np
from functools import lru_cache

LN_EPS = 1e-5
NEG_BIG = -1.0e30


def _build(n_cores, T, S, H, NH, is_pre, has_bias, repeat=1):
    import os
    import concourse.bacc as bacc
    import concourse.mybir as mybir
    import concourse.tile as tile
    from concourse.masks import make_identity

    # timing-only probe: load each weight tile once and reuse it for all
    # later chunks (outputs become garbage; never set in real use)
    probe_no_wdma = bool(int(os.environ.get("PROBE_NO_WDMA", "0")))
    _wcache = {}

    F32 = mybir.dt.float32
    F32R = mybir.dt.float32r
    BF16 = mybir.dt.bfloat16
    ACT = mybir.ActivationFunctionType
    ALU = mybir.AluOpType

    KO = H // 128          # hidden-dim 128-chunks (contraction)
    D = H // NH
    DT = D // 128          # d-chunks per head
    KT = S // 128          # key-token 128-chunks per sequence
    B_core = T // S
    TT = T // 128          # token 128-chunks
    TC = T // 512          # token 512-chunks
    NC2 = 2 * H // 128     # q+k column chunks of 128
    assert TC >= 1 and DT == 2 and KT == 2

    nc = bacc.Bacc("TRN2", target_bir_lowering=False, debug=False,
                   num_devices=n_cores)

    x_d = nc.dram_tensor("x", [T, H], F32, kind="ExternalInput")
    # host prepacks weights so each SBUF tile is one linear read:
    # qkw[c, p, ko, j]  = Wqk[ko*128+p, c*128+j]        (c < NC2)
    # vw [n, p, ko, j]  = Wv [ko*128+p, n*D+j]          (n < NH, j < D)
    # ow [c, p, ko, j]  = Wo [ko*128+p, c*128+j]        (c < KO)
    qkw_d = nc.dram_tensor("qkw", [NC2 // 2, 128, KO, 256], BF16,
                           kind="ExternalInput")
    vw_d = nc.dram_tensor("vw", [NH, 128, KO, D], BF16,
                          kind="ExternalInput")
    ow_d = nc.dram_tensor("ow", [KO // 2, 128, KO, 256], BF16,
                          kind="ExternalInput")
    # maskb[b*KT+kt, :] = additive key-mask bias for key tokens kt*128..+128
    mb_d = nc.dram_tensor("maskb", [B_core * KT, 128], F32,
                          kind="ExternalInput")
    if has_bias:
        # bqk[i, :] = (bias @ qkv)[i*128:(i+1)*128] for the q,k columns
        # (q part pre-scaled by 1/sqrt(D)); bv broadcast row for v columns.
        bqk_d = nc.dram_tensor("bqk", [NC2, 128], F32, kind="ExternalInput")
        bv_d = nc.dram_tensor("bv", [NH * 2, 128], F32,
                              kind="ExternalInput")
    if is_pre:
        out_d = nc.dram_tensor("outT", [H, T], BF16, kind="ExternalOutput")
    else:
        lnw_d = nc.dram_tensor("lnw", [H], F32, kind="ExternalInput")
        lnb_d = nc.dram_tensor("lnb", [H], F32, kind="ExternalInput")
        out_d = nc.dram_tensor("outN", [T, H], F32, kind="ExternalOutput")

    from contextlib import ExitStack

    with tile.TileContext(nc) as tc:
        with ExitStack() as stack:
            pool = lambda *a, **kw: stack.enter_context(  # noqa: E731
                tc.tile_pool(*a, **kw))
            cp = pool(name="consts", bufs=1)
            dp = pool(name="dram", bufs=1, space="DRAM")
            xp = pool(name="xld", bufs=1)
            xbp = pool(name="xnb", bufs=1)
            st = pool(name="stats", bufs=4)
            xnp = pool(name="xnt", bufs=1)
            wp = pool(name="wqk", bufs=2)
            # bufs=1 is safe for qkh/vhd: PE executes in order, so head n's
            # attention reads finish before head n+1's projection evacs.
            qkp = pool(name="qkh", bufs=1)
            vhp = pool(name="vhd", bufs=1)
            xpp = pool(name="expp", bufs=2)
            rp = pool(name="recp", bufs=2)
            oev = pool(name="oevp", bufs=1)
            cxp = pool(name="ctxp", bufs=1)
            # PSUM is 8 banks of 2KB/partition; every tile rounds up to a
            # bank: pQK 3 + pS 2 + m 1 + c 1 = 7.
            pQK = pool(name="psQK", bufs=3, space="PSUM")
            pS = pool(name="psS", bufs=2, space="PSUM")
            pMC = pool(name="psMC", bufs=1, space="PSUM")

            if not is_pre:
                ident = cp.tile([128, 128], F32)
                make_identity(nc, ident[:])
            onesb = cp.tile([128, 128], BF16)
            # f32 bit pattern 0x3F803F80 = two bf16 1.0s per 4-byte word
            nc.vector.memset(onesb[:].bitcast(F32),
                             float(np.frombuffer(
                                 np.uint32(0x3F803F80).tobytes(),
                                 dtype=np.float32)[0]))
            eps_t = cp.tile([128, 1], F32)
            nc.vector.memset(eps_t[:], LN_EPS)
            mb_t = cp.tile([128, B_core * KT], F32)
            nc.sync.dma_start(mb_t[:], mb_d.ap().rearrange("i p -> p i"))
            if has_bias:
                bqk_t = cp.tile([128, NC2], F32)
                nc.sync.dma_start(bqk_t[:], bqk_d.ap().rearrange("i p -> p i"))
                bvt_t = cp.tile([128, NH * 2], F32)
                nc.sync.dma_start(bvt_t[:], bv_d.ap().rearrange("i p -> p i"))
            if not is_pre:
                import concourse.bass as _bass
                lnw_bc = _bass.AP(tensor=lnw_d.ap().tensor, offset=0,
                                  ap=[[0, 128], [1, H]])
                lnb_bc = _bass.AP(tensor=lnb_d.ap().tensor, offset=0,
                                  ap=[[0, 128], [1, H]])
                lnw_t = cp.tile([128, H], F32)
                nc.sync.dma_start(lnw_t[:], lnw_bc)
                lnb_t = cp.tile([128, H], F32)
                nc.sync.dma_start(lnb_t[:], lnb_bc)
                oTs = dp.tile([H, T], F32)

            def _wtile(pool_, shape, tag, src_ap):
                if probe_no_wdma and tag in _wcache:
                    return _wcache[tag]
                t = pool_.tile(shape, BF16, tag=tag, name=tag)
                nc.sync.dma_start(t[:], src_ap)
                if probe_no_wdma:
                    _wcache[tag] = t
                return t

            def emit_phaseA_tt(xnT, tt):
                """LN one 128-token block; DMA-XBAR-transpose into xnT."""
                xt = xp.tile([128, H], F32, tag="xt", name="xt")
                nc.sync.dma_start(xt[:], x_d.ap()[tt * 128:(tt + 1) * 128, :])
                xnb = xbp.tile([128, H // 4], BF16, tag="xnb", name="xnb")
                if is_pre:
                    ssum = st.tile([128, 1], F32, tag="ss", name="ss")
                    nc.vector.reduce_sum(out=ssum[:], in_=xt[:],
                                         axis=mybir.AxisListType.X)
                    # sum of squares, chunked; the elementwise output is
                    # discarded scratch (xnb is overwritten below)
                    NSQ = 4
                    HC = H // NSQ
                    parts = []
                    for i in range(NSQ):
                        p = st.tile([128, 1], F32, tag=f"sq{i}",
                                    name=f"sq{i}")
                        nc.scalar.activation(
                            xnb[:], xt[:, i * HC:(i + 1) * HC],
                            ACT.Square, accum_out=p[:])
                        parts.append(p)
                    while len(parts) > 1:
                        a = parts.pop(0)
                        b_ = parts.pop(0)
                        s = st.tile([128, 1], F32, tag="sqa", name="sqa")
                        nc.vector.tensor_tensor(s[:], a[:], b_[:], ALU.add)
                        parts.append(s)
                    sumsq = parts[0]
                    negmu = st.tile([128, 1], F32, tag="nm", name="nm")
                    nc.vector.tensor_scalar_mul(negmu[:], ssum[:], -1.0 / H)
                    musq = st.tile([128, 1], F32, tag="mq", name="mq")
                    nc.vector.tensor_tensor(musq[:], negmu[:], negmu[:],
                                            ALU.mult)
                    bias_t = st.tile([128, 1], F32, tag="bt", name="bt")
                    nc.vector.tensor_tensor(bias_t[:], eps_t[:], musq[:],
                                            ALU.subtract)
                    sd = st.tile([128, 1], F32, tag="sd", name="sd")
                    nc.scalar.activation(sd[:], sumsq[:], ACT.Sqrt,
                                         bias=bias_t[:], scale=1.0 / H)
                    rstd = st.tile([128, 1], F32, tag="rs", name="rs")
                    nc.vector.reciprocal(rstd[:], sd[:])
                for h in range(4):
                    dst = xnT[:, h * (KO // 4):(h + 1) * (KO // 4),
                              tt * 128:(tt + 1) * 128]
                    if is_pre:
                        nc.vector.tensor_scalar(
                            out=xnb[:], in0=xt[:, h * (H // 4):
                                               (h + 1) * (H // 4)],
                            scalar1=negmu[:], scalar2=rstd[:],
                            op0=ALU.add, op1=ALU.mult)
                    else:
                        nc.vector.tensor_copy(
                            xnb[:], xt[:, h * (H // 4):(h + 1) * (H // 4)])
                    nc.sync.dma_start_transpose(dst, xnb[:])

            def emit_head(xnT, ctxT, n):
                # ---- q,k projections (weight-stationary) ----
                # f32r: the softmax is sharp (score std ~6), so bf16 q/k
                # storage noise (~0.01 absolute on scores) costs ~1% output
                # error; f32r storage keeps scores near-exact and runs at
                # the same PE rate for free dim >= 256.
                qT = qkp.tile([128, DT, T], F32R, tag="qT", name="qT")
                kT = qkp.tile([128, DT, T], F32R, tag="kT", name="kT")
                for which, dest in ((0, qT), (1, kT)):
                    wt = _wtile(wp, [128, KO, 256], "w",
                                qkw_d.ap()[which * (NC2 // 4) + n])
                    for dt in range(DT):
                        c = which * (H // 128) + n * DT + dt
                        for tch in range(TC):
                            ps = pQK.tile([128, 512], F32, tag="qk",
                                          name="psqk")
                            for ko in range(KO):
                                nc.tensor.matmul(
                                    ps[:], wt[:, ko, dt * 128:(dt + 1) * 128],
                                    xnT[:, ko, tch * 512:(tch + 1) * 512],
                                    start=(ko == 0), stop=(ko == KO - 1))
                            dst = dest[:, dt, tch * 512:(tch + 1) * 512]
                            if has_bias:
                                nc.vector.tensor_scalar_add(
                                    dst, ps[:], bqk_t[:, c:c + 1])
                            else:
                                nc.vector.tensor_copy(dst, ps[:])
                # ---- v projection (weight-stationary, N=512), then
                # DMA-XBAR-transpose into the [token, d] layout ----
                wv = _wtile(wp, [128, KO, D], "w", vw_d.ap()[n])
                vT = qkp.tile([128, DT, T], BF16, tag="vT", name="vT")
                for dt in range(DT):
                    for tch in range(TC):
                        ps = pQK.tile([128, 512], F32, tag="qk", name="psqk")
                        for ko in range(KO):
                            nc.tensor.matmul(
                                ps[:], wv[:, ko, dt * 128:(dt + 1) * 128],
                                xnT[:, ko, tch * 512:(tch + 1) * 512],
                                start=(ko == 0), stop=(ko == KO - 1))
                        dst = vT[:, dt, tch * 512:(tch + 1) * 512]
                        if has_bias:
                            nc.vector.tensor_scalar_add(
                                dst, ps[:], bvt_t[:, n * DT + dt:
                                                  n * DT + dt + 1])
                        else:
                            nc.vector.tensor_copy(dst, ps[:])
                vh = vhp.tile([128, TT, D], BF16, tag="vh", name="vh")
                for dt in range(DT):
                    nc.sync.dma_start_transpose(
                        vh[:, :, dt * 128:(dt + 1) * 128], vT[:, dt, :])

                # ---- attention, software-pipelined one batch ahead ----
                pss = {}
                exps = {}

                def emit_scores(b):
                    ps = pS.tile([128, 2 * S], F32, tag="s", name="pss")
                    for kt in range(KT):
                        for dt in range(DT):
                            nc.tensor.matmul(
                                ps[:, kt * S:(kt + 1) * S],
                                kT[:, dt, b * S + kt * 128:
                                   b * S + (kt + 1) * 128],
                                qT[:, dt, b * S:(b + 1) * S],
                                start=(dt == 0), stop=(dt == DT - 1))
                    ex = xpp.tile([128, KT, S], BF16, tag="ex", name="ex")
                    for kt in range(KT):
                        nc.scalar.activation(
                            ex[:, kt], ps[:, kt * S:(kt + 1) * S], ACT.Exp,
                            bias=mb_t[:, b * KT + kt:b * KT + kt + 1],
                            scale=1.0)
                    pss[b] = ps
                    exps[b] = ex

                def emit_ctx(b):
                    ex = exps.pop(b)
                    pss.pop(b)
                    psm = pMC.tile([128, S], F32, tag="m", name="psm")
                    for kt in range(KT):
                        nc.tensor.matmul(psm[:], onesb[:], ex[:, kt],
                                         start=(kt == 0), stop=(kt == KT - 1))
                    rec = rp.tile([128, S], BF16, tag="rec", name="rec")
                    with nc.allow_low_precision(reason="softmax denom bf16"):
                        nc.vector.reciprocal(rec[:], psm[:])
                    psc = pMC.tile([128, DT, S], F32, tag="c", name="psc")
                    for dt in range(DT):
                        for kt in range(KT):
                            nc.tensor.matmul(
                                psc[:, dt],
                                vh[:, 2 * b + kt, dt * 128:(dt + 1) * 128],
                                ex[:, kt, :],
                                start=(kt == 0), stop=(kt == KT - 1))
                        nc.vector.tensor_tensor(
                            ctxT[:, n * DT + dt, b * S:(b + 1) * S],
                            psc[:, dt], rec[:], ALU.mult)

                emit_scores(0)
                for b in range(B_core):
                    if b + 1 < B_core:
                        emit_scores(b + 1)
                    emit_ctx(b)

            def emit_oproj_chunk(ctxT, hoch2):
                wo = _wtile(wp, [128, KO, 256], "w", ow_d.ap()[hoch2])
                for half in range(2):
                    hoch = 2 * hoch2 + half
                    for tch in range(TC):
                        ps = pQK.tile([128, 512], F32, tag="qk", name="psqk")
                        for ko in range(KO):
                            nc.tensor.matmul(
                                ps[:], wo[:, ko, half * 128:(half + 1) * 128],
                                ctxT[:, ko, tch * 512:(tch + 1) * 512],
                                start=(ko == 0), stop=(ko == KO - 1))
                        if is_pre:
                            ev = oev.tile([128, 512], BF16, tag="ev",
                                          name="ev")
                            nc.vector.tensor_copy(ev[:], ps[:])
                            nc.sync.dma_start(
                                out_d.ap()[hoch * 128:(hoch + 1) * 128,
                                           tch * 512:(tch + 1) * 512], ev[:])
                        else:
                            ev = oev.tile([128, 512], F32, tag="ev",
                                          name="ev")
                            nc.vector.tensor_copy(ev[:], ps[:])
                            nc.sync.dma_start(
                                oTs[hoch * 128:(hoch + 1) * 128,
                                    tch * 512:(tch + 1) * 512], ev[:])

            # ---------------- main repeat loop ----------------
            xnT = xnp.tile([128, KO, T], BF16, tag="xnT", name="xnT")
            for tt in range(TT):
                emit_phaseA_tt(xnT, tt)
            for rep in range(repeat):
                ctxT = cxp.tile([128, KO, T], BF16, tag="ctxT", name="ctxT")
                for n in range(NH):
                    emit_head(xnT, ctxT, n)
                NO2 = KO // 2
                if rep + 1 < repeat:
                    # interleave O-proj with the next rep's Phase A so the
                    # LN DVE/ACT work hides under the O-proj matmuls.
                    xnT = xnp.tile([128, KO, T], BF16, tag="xnT", name="xnT")
                    for hoch2 in range(NO2):
                        emit_oproj_chunk(ctxT, hoch2)
                        if hoch2 % (NO2 // TT) == 0:
                            emit_phaseA_tt(xnT, hoch2 // (NO2 // TT))
                else:
                    for hoch2 in range(NO2):
                        emit_oproj_chunk(ctxT, hoch2)

            # ---------------- isPre=0: transpose + post-LN ----------------
            if not is_pre:
                with tc.tile_pool(name="p4in", bufs=3) as p4i, \
                     tc.tile_pool(name="p4out", bufs=2) as p4o, \
                     tc.tile_pool(name="st4", bufs=8) as st4, \
                     tc.tile_pool(name="sq4", bufs=2) as sq4, \
                     tc.tile_pool(name="tps4", bufs=4, space="PSUM") as tp4:
                    for tt in range(TT):
                        on = p4o.tile([128, H], F32)
                        for hh in range(KO):
                            it = p4i.tile([128, 128], F32)
                            nc.sync.dma_start(
                                it[:], oTs[hh * 128:(hh + 1) * 128,
                                           tt * 128:(tt + 1) * 128])
                            pt = tp4.tile([128, 128], F32)
                            nc.tensor.transpose(pt[:], it[:], ident[:])
                            nc.vector.tensor_copy(
                                on[:, hh * 128:(hh + 1) * 128], pt[:])
                        ssum = st4.tile([128, 1], F32)
                        nc.vector.reduce_sum(out=ssum[:], in_=on[:],
                                             axis=mybir.AxisListType.X)
                        negmu = st4.tile([128, 1], F32)
                        nc.vector.tensor_scalar_mul(negmu[:], ssum[:],
                                                    -1.0 / H)
                        xsq = sq4.tile([128, H], F32)
                        vsum = st4.tile([128, 1], F32)
                        nc.scalar.activation(xsq[:], on[:], ACT.Square,
                                             bias=negmu[:], scale=1.0,
                                             accum_out=vsum[:])
                        sd = st4.tile([128, 1], F32)
                        nc.scalar.activation(sd[:], vsum[:], ACT.Sqrt,
                                             bias=eps_t[:], scale=1.0 / H)
                        rstd = st4.tile([128, 1], F32)
                        nc.vector.reciprocal(rstd[:], sd[:])
                        nc.vector.tensor_scalar(
                            out=on[:], in0=on[:],
                            scalar1=negmu[:], scalar2=rstd[:],
                            op0=ALU.add, op1=ALU.mult)
                        nc.vector.tensor_tensor(on[:], on[:], lnw_t[:],
                                                ALU.mult)
                        nc.vector.tensor_tensor(on[:], on[:], lnb_t[:],
                                                ALU.add)
                        nc.sync.dma_start(
                            out_d.ap()[tt * 128:(tt + 1) * 128, :], on[:])

    nc.finalize()
    return nc


@lru_cache(maxsize=4)
def _get_runner(n_cores, T, S, H, NH, is_pre, has_bias, repeat=1):
    """Build + jit once; returns fn(in_maps) -> list of out dicts."""
    import jax
    import numpy as _np
    from jax.sharding import Mesh, PartitionSpec
    from jax.experimental.shard_map import shard_map
    import concourse.mybir as mybir
    from concourse import bass2jax
    from concourse.bass2jax import _bass_exec_p, install_neuronx_cc_hook

    nc = _build(n_cores, T, S, H, NH, is_pre, has_bias, repeat)
    install_neuronx_cc_hook()

    partition_name = (nc.partition_id_tensor.name
                      if nc.partition_id_tensor else None)
    in_names, out_names, out_avals, zero_shapes = [], [], [], []
    for alloc in nc.m.functions[0].allocations:
        if not isinstance(alloc, mybir.MemoryLocationSet):
            continue
        name = alloc.memorylocations[0].name
        if alloc.kind == "ExternalInput":
            if name != partition_name:
                in_names.append(name)
        elif alloc.kind == "ExternalOutput":
            out_names.append(name)
            shape = tuple(alloc.tensor_shape)
            dtype = mybir.dt.np(alloc.dtype)
            out_avals.append(jax.core.ShapedArray(shape, dtype))
            zero_shapes.append((shape, dtype))
    n_outs = len(out_avals)
    all_in_names = list(in_names) + list(out_names)
    if partition_name is not None:
        all_in_names.append(partition_name)

    def _body(*args):
        operands = list(args)
        if partition_name is not None:
            operands.append(bass2jax.partition_id_tensor())
        outs = _bass_exec_p.bind(
            *operands,
            out_avals=tuple(out_avals),
            in_names=tuple(all_in_names),
            out_names=tuple(out_names),
            lowering_input_output_aliases=(),
            sim_require_finite=True,
            sim_require_nnan=True,
            nc=nc,
        )
        return tuple(outs)

    devices = jax.devices()[:n_cores]
    if n_cores == 1:
        jfn = jax.jit(_body, keep_unused=True)

        def _prep(in_maps):
            args = [jax.device_put(_np.asarray(in_maps[0][n]))
                    for n in in_names]
            zeros = [jax.device_put(_np.zeros(s, d)) for s, d in zero_shapes]
            return args + zeros

        def _collect(outs):
            return [{n: _np.asarray(outs[i]) for i, n in enumerate(out_names)}]
    else:
        mesh = Mesh(np.asarray(devices), ("core",))
        from jax.sharding import NamedSharding
        shard = NamedSharding(mesh, PartitionSpec("core"))
        repl = NamedSharding(mesh, PartitionSpec())
        REPLICATED = {"qkw", "vw", "ow", "bqk", "bv", "lnw", "lnb"}
        in_specs = tuple(
            (PartitionSpec() if n in REPLICATED else PartitionSpec("core"))
            for n in in_names) + (PartitionSpec("core"),) * n_outs
        out_specs = (PartitionSpec("core"),) * n_outs
        jfn = jax.jit(
            shard_map(_body, mesh=mesh, in_specs=in_specs,
                      out_specs=out_specs, check_rep=False),
            keep_unused=True)

        def _prep(in_maps):
            concat_in = []
            for n in in_names:
                if n in REPLICATED:
                    concat_in.append(
                        jax.device_put(_np.asarray(in_maps[0][n]), repl))
                else:
                    concat_in.append(jax.device_put(
                        _np.concatenate([_np.asarray(m[n]) for m in in_maps],
                                        axis=0), shard))
            zeros = [
                jax.device_put(
                    _np.zeros((n_cores * s[0], *s[1:]), d), shard)
                for s, d in zero_shapes]
            return concat_in + zeros

        def _collect(outs):
            return [
                {n: _np.asarray(outs[i]).reshape(
                    n_cores, *out_avals[i].shape)[c]
                 for i, n in enumerate(out_names)}
                for c in range(n_cores)]

    class Runner:
        in_names_ = in_names
        out_names_ = out_names

        def prep(self, in_maps):
            return _prep(in_maps)

        def call(self, args):
            return jfn(*args)

        def run(self, in_maps):
            outs = jfn(*_prep(in_maps))
            jax.block_until_ready(outs)
            return _collect(outs)

        def collect(self, outs):
            return _collect(outs)

    return Runner()


def _prep_core_inputs(inp, mask, weight, bias, qkv, o, is_pre, n_cores,
                      NH=16):
    """Host-side prep: fold LN weight + 1/sqrt(D) into qkv, prepack the
    weights into per-tile-contiguous bf16 layouts, build per-core dicts."""
    import ml_dtypes
    BF16 = ml_dtypes.bfloat16

    B, S, H = inp.shape
    D = H // NH
    B_core = B // n_cores
    T = B_core * S
    KO = H // 128
    KT = S // 128
    NC2 = 2 * H // 128

    # Pre-LN: xn = z*w + b with z the normalized input, so
    # xn @ qkv = z @ (w[:,None]*qkv) + (b @ qkv): fold w into the weights
    # and b into per-column additive terms applied on-device. The
    # 1/sqrt(D) query scale is folded into the q weight columns.
    qkvw = qkv.astype(np.float32)
    if is_pre:
        w = weight.astype(np.float32)
        if not np.all(w == 1.0):
            qkvw = qkvw * w[:, None]
        bfull = bias.astype(np.float32) @ qkv.astype(np.float32)
    else:
        bfull = np.zeros(3 * H, dtype=np.float32)
    qsc = np.float32(1.0 / np.sqrt(D))
    qkvw = qkvw.copy()
    qkvw[:, :H] *= qsc
    bfull = bfull.copy()
    bfull[:H] *= qsc
    has_bias = bool(np.any(bfull))

    # prepack: tile [128, KO, 256] contiguous per merged pair-chunk index
    wqk = qkvw[:, :2 * H].reshape(KO, 128, NC2 // 2, 256)
    wqk = np.ascontiguousarray(wqk.transpose(2, 1, 0, 3)).astype(BF16)
    wv = qkvw[:, 2 * H:].reshape(KO, 128, NH, D)
    wv = np.ascontiguousarray(wv.transpose(2, 1, 0, 3)).astype(BF16)
    ow = o.astype(np.float32).reshape(KO, 128, KO // 2, 256)
    ow = np.ascontiguousarray(ow.transpose(2, 1, 0, 3)).astype(BF16)

    maskbias = np.where(mask != 0, np.float32(NEG_BIG), np.float32(0.0))
    maskbias = maskbias.astype(np.float32)  # [B, S]

    in_maps = []
    for c in range(n_cores):
        xb = inp[c * B_core:(c + 1) * B_core].reshape(T, H)
        mb = maskbias[c * B_core:(c + 1) * B_core].reshape(B_core * KT, 128)
        m = {
            "x": np.ascontiguousarray(xb.astype(np.float32)),
            "qkw": wqk,
            "vw": wv,
            "ow": ow,
            "maskb": np.ascontiguousarray(mb),
        }
        if has_bias:
            m["bqk"] = np.ascontiguousarray(
                bfull[:2 * H].reshape(NC2, 128))
            m["bv"] = np.ascontiguousarray(
                bfull[2 * H:].reshape(NH * 2, 128))
        if not is_pre:
            m["lnw"] = np.ascontiguousarray(weight.astype(np.float32))
            m["lnb"] = np.ascontiguousarray(bias.astype(np.float32))
        in_maps.append(m)
    return in_maps, has_bias, (B, S, H, NH, B_core, T)


def kernel(inp, mask, weight, bias, qkv, o, isPre):
    inp = np.asarray(inp)
    mask = np.asarray(mask)
    weight = np.asarray(weight)
    bias = np.asarray(bias)
    qkv = np.asarray(qkv)
    o = np.asarray(o)
    is_pre = bool(int(np.asarray(isPre)))

    n_cores = 8
    NH = 16
    in_maps, has_bias, (B, S, H, _, B_core, T) = _prep_core_inputs(
        inp, mask, weight, bias, qkv, o, is_pre, n_cores)

    runner = _get_runner(n_cores, T, S, H, NH, is_pre, has_bias)
    results = runner.run(in_maps)

    out = np.empty((B, S, H), dtype=np.float32)
    for c in range(n_cores):
        if is_pre:
            outT = results[c]["outT"].astype(np.float32)  # [H, T]
            out[c * B_core:(c + 1) * B_core] = outT.T.reshape(B_core, S, H)
        else:
            out[c * B_core:(c + 1) * B_core] = (
                results[c]["outN"].reshape(B_core, S, H))
    return out
